# revision 26
# baseline (speedup 1.0000x reference)
"""HSTU layer kernel for Trainium2, 8 NeuronCores.

Sharding: core = 4*b + hg  (b in {0,1} data-parallel over batch,
hg in {0..3} head-parallel: 4 heads = 256 channels of U/V/Q/K each).

v6 @112us (baseline 220us). Key techniques, in order of impact:
  - bf16 operands everywhere (tolerance 2e-2; psum/stats stay f32),
    full 128-partition contraction/output packing on every matmul,
    causal-trimmed matmul widths in logits/AV
  - mask is Toeplitz in (i-j): resident [128, 4, 1024] window table
    replaces the 16MB streamed dense mask; key padding handled by
    zeroing V rows (silu scale=0); alpha folded into the mask-add
  - PE kept dense so the HAM clock gate stays at 2.4GHz: head-pair
    interleaved, depth-4 software-pipelined attention; out_proj matmul
    groups popped from a work queue inside the attention loops; Q-wave
    in_proj runs dc-outer so it pipelines with the input DMA stream
  - V stored in a 3-block [head_even | zeros | head_odd] layout so the
    AV matmul lhsT spans 128 free cols (psum output at full partitions,
    base 0 -- a psum write at column offset 64 crashes the HW)
  - DMA issue split across both HWDGE engines (sync + scalar), small
    inputs packed into one [128, 1298] tensor, bf16 outputs
  - stats/nUA elementwise work spread over DVE + Pool

Per core (channels-on-partitions, zero device transposes):
  qk^T = silu(W_qk @ x^T + b)            [128, 4 ct, t] bf16
  U    = silu(W_u @ x^T + b)             [128, 2 g, t] bf16; gU = gamma*U
  V    = silu(x @ W_v^T + b_v) * vmask   3-block layout, bf16
  logits^T[j,i] = K_h @ Q_h^T            (psum f32)
  att^T = silu(alpha*logits^T + MT_win)  bf16   (MT Toeplitz window table)
  AVs^T_h = (V_h^T @ att^T) * invd       [128, 2 g, t] f32r
  s1 = sum_c AVs, s2 = sum_c AVs^2       (ones-matmul, contraction 128)
  A^T = WO^T @ (AVs*gU)^T ; B^T = WO^T @ gU^T   (out_proj partials)

Host combine (LayerNorm is linear in its input given row stats):
  y = rho*A - (rho*mu)*B (+ C) + b_out + x
"""
import math
import numpy as np
import ml_dtypes

import concourse.bass as bass
import concourse.mybir as mybir
import concourse.tile as tile
from concourse import bacc
from concourse.bass_utils import run_bass_kernel_spmd

NUM_HEADS = 16
NUM_BUCKETS = 32
MAX_DISTANCE = 128
NEG_INF = -1e9
LN_EPS = 1e-5

B, T, D = 2, 1024, 1024
H_PER = 4           # heads per core
F32R = mybir.dt.float32r
F32 = mybir.dt.float32
BF16 = mybir.dt.bfloat16
BF16_NP = ml_dtypes.bfloat16

# packed-small-input column offsets
PK_BQK, PK_BU, PK_GAM, PK_BET = 0, 4, 6, 8
PK_VM, PK_INVD, PK_BVB, PK_N = 10, 18, 1042, 1298

LAST_RESULTS = None


def _bucket_np(n):
    """T5-style log bucket for clamped distance n >= 0."""
    max_exact = NUM_BUCKETS // 2
    with np.errstate(divide="ignore", invalid="ignore"):
        large = max_exact + (
            np.log(n.astype(np.float32) / max_exact + 1e-6)
            / math.log(MAX_DISTANCE / max_exact)
            * (NUM_BUCKETS - max_exact)
        ).astype(np.int32)
    large = np.minimum(large, NUM_BUCKETS - 1)
    return np.where(n < max_exact, n, large)


def _toeplitz_mask_np(rab_emb, heads):
    """MT[p, jj, c] = rab_h(c - p) for c-p >= 0 else -1e9.  [128, 4, 1024]"""
    d = np.arange(T)[None, :] - np.arange(128)[:, None]   # [128, 1024] = c - p
    n = np.clip(d, 0, None)
    buckets = _bucket_np(n)                               # [128, 1024]
    out = np.empty((128, len(heads), T), np.float32)
    for jj, h in enumerate(heads):
        out[:, jj, :] = np.where(d < 0, NEG_INF, rab_emb[buckets, h])
    return np.ascontiguousarray(out)


def _build(with_c):
    nc = bacc.Bacc("TRN2", target_bir_lowering=False, debug=False, num_devices=8)

    def inp(name, shape, dt):
        return nc.dram_tensor(name, shape, dt, kind="ExternalInput").ap()

    XC = inp("XC", [8, 128, 1024], BF16)     # x[b].T d-chunks
    WC = inp("WC", [8, 128, 768], BF16)      # W_in.T d-chunks: [U(256)|Q(256)|K(256)]
    WVC = inp("WVC", [2, 128, 4, 256], BF16)  # W_v.T d-chunks, 2 bundles
    WO = inp("WO", [128, 2, 1024], BF16)     # W_out cols slice: [c%128, c//128, dout]
    MT = inp("MT", [128, 4, 1024], BF16)     # Toeplitz mask windows per local head
    PK = inp("PK", [128, PK_N], F32)         # packed biases/gamma/beta/vmask/invd/bvb
    ONESP = inp("ONESP", [128, 1], F32R)

    AT = nc.dram_tensor("AT", [8, 128, 1024], BF16, kind="ExternalOutput").ap()
    BT = nc.dram_tensor("BT", [8, 128, 1024], BF16, kind="ExternalOutput").ap()
    CT = (nc.dram_tensor("CT", [8, 128, 1024], BF16, kind="ExternalOutput").ap()
          if with_c else None)
    SOUT = nc.dram_tensor("SOUT", [1, 4, 512], F32, kind="ExternalOutput").ap()

    alpha = (D // NUM_HEADS) ** (-0.5)
    SILU = mybir.ActivationFunctionType.Silu
    IDENT = mybir.ActivationFunctionType.Identity
    MULT = mybir.AluOpType.mult
    ADD = mybir.AluOpType.add

    with tile.TileContext(nc) as tc:
        with (
            tc.tile_pool(name="big", bufs=1) as big,
            tc.tile_pool(name="scratch", bufs=1) as scratch,
            tc.tile_pool(name="apool", bufs=6) as apool,
            tc.tile_pool(name="opool", bufs=3) as opool,
            tc.tile_pool(name="ps", bufs=5, space="PSUM") as ps,
            tc.tile_pool(name="psav", bufs=2, space="PSUM") as psav,
            tc.tile_pool(name="psst", bufs=1, space="PSUM") as psst,
        ):
            # ---- resident loads: x chunks on sync, W chunks on scalar so
            # issue serialization doesn't gate the in_proj start ----
            xcs, wcs = [], []
            for dc in range(4):
                xt = big.tile([128, 1024], BF16, tag=f"xc{dc}")
                nc.sync.dma_start(xt[:], XC[dc])
                xcs.append(xt)
            PKs = big.tile([128, PK_N], F32)
            nc.sync.dma_start(PKs[:], PK[:])
            for dc in range(4, 8):
                xt = big.tile([128, 1024], BF16, tag=f"xc{dc}")
                nc.sync.dma_start(xt[:], XC[dc])
                xcs.append(xt)
            for dc in range(8):
                wt = big.tile([128, 768], BF16, tag=f"wc{dc}")
                nc.scalar.dma_start(wt[:], WC[dc])
                wcs.append(wt)
            wv2 = []
            for bi in range(2):
                wv = big.tile([128, 4, 256], BF16, tag=f"wv{bi}")
                (nc.sync if bi == 0 else nc.scalar).dma_start(wv[:], WVC[bi])
                wv2.append(wv)
            WOs = big.tile([128, 2, 1024], BF16)
            nc.scalar.dma_start(WOs[:], WO[:])
            MTs = big.tile([128, 4, 1024], BF16)
            nc.sync.dma_start(MTs[:], MT[:])
            ONESs = big.tile([128, 1], F32R)
            nc.scalar.dma_start(ONESs[:], ONESP[:])

            # ---- in_proj Q,K transposed: qk = silu(W @ x^T + b) bf16 ----
            # Q wave runs dc-OUTER over 4 live psum groups: each x/W chunk is
            # consumed for all groups as soon as its DMA lands, so the PE
            # pipelines with the input stream instead of stalling on chunk 7.
            qk = big.tile([128, 4, 1024], BF16)  # ct 0,1=Q  2,3=K
            for wave in range(2):                # wave 0 = Q (ct 0,1), 1 = K
                pts = [ps.tile([128, 512], F32, tag="mm", name=f"ptw{wave}_{i}")
                       for i in range(4)]
                for dc in range(8):
                    for gi, (ct, th) in enumerate(
                            [(2 * wave + c, t) for c in range(2)
                             for t in range(2)]):
                        nc.tensor.matmul(
                            pts[gi][:],
                            wcs[dc][:, 256 + ct * 128:256 + (ct + 1) * 128],
                            xcs[dc][:, th * 512:(th + 1) * 512],
                            start=(dc == 0), stop=(dc == 7),
                        )
                for gi, (ct, th) in enumerate(
                        [(2 * wave + c, t) for c in range(2)
                         for t in range(2)]):
                    nc.scalar.activation(
                        qk[:, ct, th * 512:(th + 1) * 512], pts[gi][:],
                        SILU, bias=PKs[:, PK_BQK + ct:PK_BQK + ct + 1],
                        scale=1.0,
                    )

            # ---- in_proj U packed [128, 2 groups, t] ----
            U128 = big.tile([128, 2, 1024], BF16)
            for g in range(2):
                for th in range(2):
                    pt = ps.tile([128, 512], F32, tag="mm")
                    for dc in range(8):
                        nc.tensor.matmul(
                            pt[:],
                            wcs[dc][:, g * 128:(g + 1) * 128],
                            xcs[dc][:, th * 512:(th + 1) * 512],
                            start=(dc == 0), stop=(dc == 7),
                        )
                    nc.scalar.activation(
                        U128[:, g, th * 512:(th + 1) * 512], pt[:],
                        SILU, bias=PKs[:, PK_BU + g:PK_BU + g + 1], scale=1.0,
                    )

            # ---- beta*U (pre-gamma) if needed, then gU = gamma*U ----
            if with_c:
                bU = scratch.tile([128, 2, 1024], BF16, tag="bu")
                for g in range(2):
                    nc.vector.tensor_scalar_mul(
                        bU[:, g, :], U128[:, g, :],
                        PKs[:, PK_BET + g:PK_BET + g + 1])
            gU = scratch.tile([128, 2, 1024], BF16, tag="gu")
            for g in range(2):
                nc.vector.tensor_scalar_mul(
                    gU[:, g, :], U128[:, g, :],
                    PKs[:, PK_GAM + g:PK_GAM + g + 1])

            # ---- out_proj emitters; interleaved into the attention loops so
            # the in-order PE always has independent matmuls queued while a
            # tile's mask+silu chain is in flight (keeps HAM at full clock)
            def make_bgroup(dst, rhs, dt_, nm):
                def emit():
                    stb = opool.tile([128, 1024], BF16, tag="stB",
                                     name=f"stb_{nm}_{dt_}")
                    for th in range(2):
                        pt = ps.tile([128, 512], F32, tag="mm",
                                     name=f"bpt_{nm}_{dt_}_{th}")
                        for g in range(2):
                            nc.tensor.matmul(
                                pt[:],
                                WOs[:, g, dt_ * 128:(dt_ + 1) * 128],
                                rhs[:, g, th * 512:(th + 1) * 512],
                                start=(g == 0), stop=(g == 1),
                            )
                        nc.vector.tensor_copy(
                            out=stb[:, th * 512:(th + 1) * 512], in_=pt[:])
                    nc.sync.dma_start(dst[dt_], stb[:])
                return emit

            def make_agroup(nUA, dt_, rt):
                half = slice(rt * 512, (rt + 1) * 512)

                def emit():
                    pt = ps.tile([128, 512], F32, tag="mm",
                                 name=f"apt{rt}_{dt_}")
                    for g in range(2):
                        nc.tensor.matmul(
                            pt[:],
                            WOs[:, g, dt_ * 128:(dt_ + 1) * 128],
                            nUA[:, g, half],
                            start=(g == 0), stop=(g == 1),
                        )
                    st = opool.tile([128, 512], BF16, tag="stA",
                                    name=f"sta{rt}_{dt_}")
                    if dt_ % 2 == 0:
                        nc.scalar.activation(st[:], pt[:], IDENT)
                    else:
                        nc.vector.tensor_copy(out=st[:], in_=pt[:])
                    nc.sync.dma_start(AT[dt_, :, half], st[:])
                return emit

            extra_work = [make_bgroup(BT, gU, dt_, "b") for dt_ in range(8)]
            if with_c:
                extra_work += [make_bgroup(CT, bU, dt_, "c")
                               for dt_ in range(8)]

            # ---- in_proj natural: V = silu(x @ W_V^T + b_V) * vmask ----
            # V3 [t%128, t//128, pair, 3 blocks, 64]: head 2g in block 0,
            # head 2g+1 in block 2, block 1 stays zero.  AV lhsT for head j
            # is blocks (j%2) : (j%2)+2 -> 128 free cols, half zeros, so the
            # psum output lands at full partitions with head j's channels at
            # partition base (j%2)*64.
            V3 = big.tile([128, 8, 2, 3, 64], BF16)
            nc.gpsimd.memset(V3[:], 0.0)
            for tt in range(8):
                pt = ps.tile([128, 512], F32, tag="mm")
                for dc in range(8):
                    nc.tensor.matmul(
                        pt[:, :256],
                        xcs[dc][:, tt * 128:(tt + 1) * 128],
                        wv2[dc // 4][:, dc % 4, :],
                        start=(dc == 0), stop=(dc == 7),
                    )
                vs = apool.tile([128, 256], F32, tag="vs")
                nc.vector.tensor_add(vs[:], pt[:, :256],
                                     PKs[:, PK_BVB:PK_BVB + 256])
                for g in range(2):
                    nc.scalar.activation(
                        V3[:, tt, g, 0:3:2, :],
                        vs[:, g * 128:(g + 1) * 128], SILU,
                        scale=PKs[:, PK_VM + tt:PK_VM + tt + 1])

            # ---- attention: one global software pipeline over every
            # (rt, pair, kt) step.  A single pend queue spans pair and rt
            # boundaries so the PE never drains; each head-pair shares one
            # psum accumulator (even head starts the bank, odd head
            # accumulates onto it -- its unused partition half adds zeros).
            AVs = big.tile([128, 2, 1024], F32R)  # [c%128, c//128, t]
            nUA = scratch.tile([128, 2, 1024], BF16, tag="nua")
            sstage = scratch.tile([1, 4, 512], F32, tag="sst")
            sqhs = []
            pend = []
            done_in_rt = [0, 0]

            def rt_epilogue(rt_):
                # nUA per g on DVE, sq on Pool; A-proj groups queued as
                # extra_work so they drain inside the remaining attention
                half = slice(rt_ * 512, (rt_ + 1) * 512)
                sqh = scratch.tile([128, 2, 512], F32R, tag=f"sqh{rt_}",
                                   name=f"sqh{rt_}")
                sqhs.append(sqh)
                nc.gpsimd.tensor_mul(sqh[:], AVs[:, :, half],
                                     AVs[:, :, half])
                for g in range(2):
                    nc.vector.tensor_mul(nUA[:, g, half], AVs[:, g, half],
                                         gU[:, g, half])
                extra_work.extend(
                    make_agroup(nUA, dt_, rt_) for dt_ in range(8))

            def flush_one():
                j_, rt_, att_, off_, kt_, avp_, nkt_ = pend.pop(0)
                nc.tensor.matmul(
                    avp_[:, off_:512],
                    V3[:, kt_, j_ // 2, (j_ % 2):(j_ % 2) + 2, :],
                    att_[:, off_:512],
                    start=(kt_ == 0 and j_ % 2 == 0),
                    stop=(kt_ == nkt_ - 1 and j_ % 2 == 1),
                    skip_group_check=True,
                )
                if kt_ == nkt_ - 1 and j_ % 2 == 1:
                    # pair complete: scale by invd into AVs (both heads at
                    # once -- they occupy disjoint partition halves)
                    ch_ = j_ // 2
                    nc.vector.tensor_mul(
                        AVs[:, ch_, rt_ * 512:(rt_ + 1) * 512],
                        avp_[:],
                        PKs[:, PK_INVD + rt_ * 512:PK_INVD + (rt_ + 1) * 512])
                    done_in_rt[rt_] += 1
                    if done_in_rt[rt_] == 2:
                        rt_epilogue(rt_)

            for rt in range(2):
                n_kt = 4 * rt + 4
                for jp in (0, 2):
                    avp = psav.tile([128, 512], F32, tag="av",
                                    name=f"avp{rt}_{jp}")
                    for kt in range(n_kt):
                        d0 = rt * 512 - kt * 128
                        off = max(0, -d0)   # causal-trim: i >= kt*128
                        cs = max(0, d0)
                        for j in (jp, jp + 1):
                            pb = (j % 2) * 64
                            ch = j // 2
                            qkp = ps.tile([128, 512], F32, tag="mm",
                                          name=f"qkp{rt}_{j}_{kt}")
                            nc.tensor.matmul(
                                qkp[:, off:512],
                                qk[pb:pb + 64, 2 + ch,
                                   kt * 128:(kt + 1) * 128],
                                qk[pb:pb + 64, ch,
                                   rt * 512 + off:(rt + 1) * 512],
                                start=True, stop=True,
                            )
                            # alpha*logits + mask window -> SBUF
                            asum = apool.tile([128, 512], F32, tag="asum")
                            nc.vector.scalar_tensor_tensor(
                                asum[:, off:512], qkp[:, off:512], alpha,
                                MTs[:, j, cs:cs + 512 - off], MULT, ADD)
                            att = apool.tile([128, 512], BF16, tag="att")
                            nc.scalar.activation(att[:, off:512],
                                                 asum[:, off:512], SILU)
                            pend.append((j, rt, att, off, kt, avp, n_kt))
                            if len(pend) > 4:
                                flush_one()
                            if extra_work:
                                extra_work.pop(0)()
            while pend:
                flush_one()

            while extra_work:
                extra_work.pop(0)()

            # stats (tiny PE work, after all projection matmuls)
            for rt in range(2):
                half = slice(rt * 512, (rt + 1) * 512)
                for si in range(2):
                    sp = psst.tile([1, 512], F32, tag="st")
                    for g in range(2):
                        rhs = (AVs[:, g, half] if si == 0
                               else sqhs[rt][:, g, :])
                        nc.tensor.matmul(
                            sp[:], ONESs[:], rhs,
                            start=(g == 0), stop=(g == 1),
                        )
                    nc.vector.tensor_copy(
                        out=sstage[:, si * 2 + rt, :], in_=sp[:])

            nc.sync.dma_start(SOUT[:], sstage[:])

    nc.compile()
    return nc


_NC_CACHE = {}


def _prep_in_maps(inputs):
    x = np.asarray(inputs["x"], np.float32)
    key_padding_mask = np.asarray(inputs["key_padding_mask"])
    W_in = np.asarray(inputs["W_in"], np.float32)
    b_in = np.asarray(inputs["b_in"], np.float32)
    W_out = np.asarray(inputs["W_out"], np.float32)
    gamma = np.asarray(inputs["gamma"], np.float32)
    beta = np.asarray(inputs["beta"], np.float32)
    rab_emb = np.asarray(inputs["rab_emb"], np.float32)

    lengths = (~key_padding_mask).sum(axis=1)  # valid keys per batch
    in_maps = []
    for core in range(8):
        b, hg = core // 4, core % 4
        sl = slice(hg * 256, hg * 256 + 256)
        Wu = W_in[0:1024][sl]
        Wv = W_in[1024:2048][sl]
        Wq = W_in[2048:3072][sl]
        Wk = W_in[3072:4096][sl]
        WC_np = np.concatenate([Wu, Wq, Wk], 0).T.reshape(8, 128, 768)
        WVC_np = Wv.T.reshape(2, 4, 128, 256).transpose(0, 2, 1, 3)
        XC_np = x[b].T.reshape(8, 128, 1024)
        WO_np = np.ascontiguousarray(
            W_out[:, sl].T.reshape(2, 128, 1024).transpose(1, 0, 2))
        L = int(lengths[b])
        denom = np.clip(np.minimum(np.arange(T) + 1, L), 1, None)
        heads = [4 * hg + jj for jj in range(H_PER)]
        MT_np = _toeplitz_mask_np(rab_emb, heads)

        PK_np = np.zeros((128, PK_N), np.float32)
        bqk = np.concatenate([b_in[2048:3072][sl], b_in[3072:4096][sl]])
        PK_np[:, PK_BQK:PK_BQK + 4] = bqk.reshape(4, 128).T
        PK_np[:, PK_BU:PK_BU + 2] = b_in[0:1024][sl].reshape(2, 128).T
        PK_np[:, PK_GAM:PK_GAM + 2] = gamma[sl].reshape(2, 128).T
        PK_np[:, PK_BET:PK_BET + 2] = beta[sl].reshape(2, 128).T
        PK_np[:, PK_VM:PK_VM + 8] = (
            np.arange(128)[:, None] + 128 * np.arange(8)[None, :] < L)
        PK_np[:, PK_INVD:PK_INVD + 1024] = (1.0 / denom)[None, :]
        PK_np[:, PK_BVB:PK_BVB + 256] = b_in[1024:2048][sl][None, :]

        in_maps.append({
            "XC": np.ascontiguousarray(XC_np).astype(BF16_NP),
            "WC": np.ascontiguousarray(WC_np).astype(BF16_NP),
            "WVC": np.ascontiguousarray(WVC_np).astype(BF16_NP),
            "WO": WO_np.astype(BF16_NP),
            "MT": MT_np.astype(BF16_NP),
            "PK": PK_np,
            "ONESP": np.ones((128, 1), np.float32),
        })
    return in_maps


def kernel(x, attention_mask, key_padding_mask, W_in, b_in, W_out, b_out,
           gamma, beta, rab_emb):
    global LAST_RESULTS
    x = np.asarray(x, np.float32)
    key_padding_mask = np.asarray(key_padding_mask)
    b_out = np.asarray(b_out, np.float32)
    beta = np.asarray(beta, np.float32)

    with_c = bool(np.any(beta != 0.0))
    if with_c not in _NC_CACHE:
        _NC_CACHE[with_c] = _build(with_c)
    nc = _NC_CACHE[with_c]

    in_maps = _prep_in_maps(dict(
        x=x, attention_mask=attention_mask, key_padding_mask=key_padding_mask,
        W_in=W_in, b_in=b_in, W_out=W_out, b_out=b_out, gamma=gamma,
        beta=beta, rab_emb=rab_emb))

    res = run_bass_kernel_spmd(nc, in_maps, list(range(8)))
    LAST_RESULTS = res

    out = np.empty((B, T, D), np.float32)
    for b in range(B):
        A = np.zeros((T, D), np.float64)
        Bm = np.zeros((T, D), np.float64)
        Cm = np.zeros((T, D), np.float64)
        s1 = np.zeros(T, np.float64)
        s2 = np.zeros(T, np.float64)
        for hg in range(4):
            r = res.results[4 * b + hg]
            A += r["AT"].reshape(1024, 1024).T.astype(np.float64)
            Bm += r["BT"].reshape(1024, 1024).T.astype(np.float64)
            if with_c:
                Cm += r["CT"].reshape(1024, 1024).T.astype(np.float64)
            s = r["SOUT"].reshape(4, 512)
            s1 += np.concatenate([s[0], s[1]]).astype(np.float64)
            s2 += np.concatenate([s[2], s[3]]).astype(np.float64)
        # s1, s2 already invd-scaled on device
        mu = s1 / D
        var = s2 / D - mu * mu
        rho = 1.0 / np.sqrt(var + LN_EPS)
        y = (rho[:, None] * A - (rho * mu)[:, None] * Bm + Cm
             + b_out[None, :].astype(np.float64) + x[b].astype(np.float64))
        out[b] = y.astype(np.float32)
    return out


# revision 27
# speedup vs baseline: 1.1361x; 1.1361x over previous
"""HSTU layer kernel for Trainium2, 8 NeuronCores.

Sharding: core = 4*b + hg  (b in {0,1} data-parallel over batch,
hg in {0..3} head-parallel: 4 heads = 256 channels of U/V/Q/K each).

v6 @112us (baseline 220us). Key techniques, in order of impact:
  - bf16 operands everywhere (tolerance 2e-2; psum/stats stay f32),
    full 128-partition contraction/output packing on every matmul,
    causal-trimmed matmul widths in logits/AV
  - mask is Toeplitz in (i-j): resident [128, 4, 1024] window table
    replaces the 16MB streamed dense mask; key padding handled by
    zeroing V rows (silu scale=0); alpha folded into the mask-add
  - PE kept dense so the HAM clock gate stays at 2.4GHz: head-pair
    interleaved, depth-4 software-pipelined attention; out_proj matmul
    groups popped from a work queue inside the attention loops; Q-wave
    in_proj runs dc-outer so it pipelines with the input DMA stream
  - V stored in a 3-block [head_even | zeros | head_odd] layout so the
    AV matmul lhsT spans 128 free cols (psum output at full partitions,
    base 0 -- a psum write at column offset 64 crashes the HW)
  - DMA issue split across both HWDGE engines (sync + scalar), small
    inputs packed into one [128, 1298] tensor, bf16 outputs
  - stats/nUA elementwise work spread over DVE + Pool

Per core (channels-on-partitions, zero device transposes):
  qk^T = silu(W_qk @ x^T + b)            [128, 4 ct, t] bf16
  U    = silu(W_u @ x^T + b)             [128, 2 g, t] bf16; gU = gamma*U
  V    = silu(x @ W_v^T + b_v) * vmask   3-block layout, bf16
  logits^T[j,i] = K_h @ Q_h^T            (psum f32)
  att^T = silu(alpha*logits^T + MT_win)  bf16   (MT Toeplitz window table)
  AVs^T_h = (V_h^T @ att^T) * invd       [128, 2 g, t] f32r
  s1 = sum_c AVs, s2 = sum_c AVs^2       (ones-matmul, contraction 128)
  A^T = WO^T @ (AVs*gU)^T ; B^T = WO^T @ gU^T   (out_proj partials)

Host combine (LayerNorm is linear in its input given row stats):
  y = rho*A - (rho*mu)*B (+ C) + b_out + x
"""
import math
import numpy as np
import ml_dtypes

import concourse.bass as bass
import concourse.mybir as mybir
import concourse.tile as tile
from concourse import bacc
from concourse.bass_utils import run_bass_kernel_spmd

NUM_HEADS = 16
NUM_BUCKETS = 32
MAX_DISTANCE = 128
NEG_INF = -1e9
LN_EPS = 1e-5

B, T, D = 2, 1024, 1024
H_PER = 4           # heads per core
F32R = mybir.dt.float32r
F32 = mybir.dt.float32
BF16 = mybir.dt.bfloat16
BF16_NP = ml_dtypes.bfloat16

# packed-small-input column offsets
PK_BQK, PK_BU, PK_GAM, PK_BET = 0, 4, 6, 8
PK_VM, PK_INVD, PK_BVB, PK_N = 10, 18, 1042, 1298

LAST_RESULTS = None


def _bucket_np(n):
    """T5-style log bucket for clamped distance n >= 0."""
    max_exact = NUM_BUCKETS // 2
    with np.errstate(divide="ignore", invalid="ignore"):
        large = max_exact + (
            np.log(n.astype(np.float32) / max_exact + 1e-6)
            / math.log(MAX_DISTANCE / max_exact)
            * (NUM_BUCKETS - max_exact)
        ).astype(np.int32)
    large = np.minimum(large, NUM_BUCKETS - 1)
    return np.where(n < max_exact, n, large)


def _toeplitz_mask_np(rab_emb, heads):
    """MT[p, jj, c] = rab_h(c - p) for c-p >= 0 else -1e9.  [128, 4, 1024]"""
    d = np.arange(T)[None, :] - np.arange(128)[:, None]   # [128, 1024] = c - p
    n = np.clip(d, 0, None)
    buckets = _bucket_np(n)                               # [128, 1024]
    out = np.empty((128, len(heads), T), np.float32)
    for jj, h in enumerate(heads):
        out[:, jj, :] = np.where(d < 0, NEG_INF, rab_emb[buckets, h])
    return np.ascontiguousarray(out)


def _build(with_c):
    nc = bacc.Bacc("TRN2", target_bir_lowering=False, debug=False, num_devices=8)

    def inp(name, shape, dt):
        return nc.dram_tensor(name, shape, dt, kind="ExternalInput").ap()

    XC = inp("XC", [8, 128, 1024], BF16)     # x[b].T d-chunks
    WC = inp("WC", [8, 128, 768], BF16)      # W_in.T d-chunks: [U(256)|Q(256)|K(256)]
    WVC = inp("WVC", [2, 128, 4, 256], BF16)  # W_v.T d-chunks, 2 bundles
    WO = inp("WO", [128, 2, 1024], BF16)     # W_out cols slice: [c%128, c//128, dout]
    MT = inp("MT", [128, 4, 1024], BF16)     # Toeplitz mask windows per local head
    PK = inp("PK", [128, PK_N], F32)         # packed biases/gamma/beta/vmask/invd/bvb
    ONESP = inp("ONESP", [128, 1], F32R)

    AT = nc.dram_tensor("AT", [8, 128, 1024], BF16, kind="ExternalOutput").ap()
    BT = nc.dram_tensor("BT", [8, 128, 1024], BF16, kind="ExternalOutput").ap()
    CT = (nc.dram_tensor("CT", [8, 128, 1024], BF16, kind="ExternalOutput").ap()
          if with_c else None)
    SOUT = nc.dram_tensor("SOUT", [1, 4, 512], F32, kind="ExternalOutput").ap()

    alpha = (D // NUM_HEADS) ** (-0.5)
    SILU = mybir.ActivationFunctionType.Silu
    IDENT = mybir.ActivationFunctionType.Identity
    MULT = mybir.AluOpType.mult
    ADD = mybir.AluOpType.add

    with tile.TileContext(nc) as tc:
        with (
            tc.tile_pool(name="big", bufs=1) as big,
            tc.tile_pool(name="scratch", bufs=1) as scratch,
            tc.tile_pool(name="apool", bufs=6) as apool,
            tc.tile_pool(name="opool", bufs=3) as opool,
            tc.tile_pool(name="ps", bufs=5, space="PSUM") as ps,
            tc.tile_pool(name="psav", bufs=2, space="PSUM") as psav,
            tc.tile_pool(name="psst", bufs=1, space="PSUM") as psst,
        ):
            # ---- resident loads: x chunks on sync, W chunks on scalar so
            # issue serialization doesn't gate the in_proj start ----
            xcs, wcs = [], []
            for dc in range(4):
                xt = big.tile([128, 1024], BF16, tag=f"xc{dc}")
                nc.sync.dma_start(xt[:], XC[dc])
                xcs.append(xt)
            PKs = big.tile([128, PK_N], F32)
            nc.sync.dma_start(PKs[:], PK[:])
            for dc in range(4, 8):
                xt = big.tile([128, 1024], BF16, tag=f"xc{dc}")
                nc.sync.dma_start(xt[:], XC[dc])
                xcs.append(xt)
            for dc in range(8):
                wt = big.tile([128, 768], BF16, tag=f"wc{dc}")
                nc.scalar.dma_start(wt[:], WC[dc])
                wcs.append(wt)
            wv2 = []
            for bi in range(2):
                wv = big.tile([128, 4, 256], BF16, tag=f"wv{bi}")
                (nc.sync if bi == 0 else nc.scalar).dma_start(wv[:], WVC[bi])
                wv2.append(wv)
            WOs = big.tile([128, 2, 1024], BF16)
            nc.scalar.dma_start(WOs[:], WO[:])
            MTs = big.tile([128, 4, 1024], BF16)
            nc.sync.dma_start(MTs[:], MT[:])
            ONESs = big.tile([128, 1], F32R)
            nc.scalar.dma_start(ONESs[:], ONESP[:])

            # ---- in_proj Q,K transposed: qk = silu(W @ x^T + b) bf16 ----
            # Q wave runs dc-OUTER over 4 live psum groups: each x/W chunk is
            # consumed for all groups as soon as its DMA lands, so the PE
            # pipelines with the input stream instead of stalling on chunk 7.
            qk = big.tile([128, 4, 1024], BF16)  # ct 0,1=Q  2,3=K
            for wave in range(2):                # wave 0 = Q (ct 0,1), 1 = K
                pts = [ps.tile([128, 512], F32, tag="mm", name=f"ptw{wave}_{i}")
                       for i in range(4)]
                for dc in range(8):
                    for gi, (ct, th) in enumerate(
                            [(2 * wave + c, t) for c in range(2)
                             for t in range(2)]):
                        nc.tensor.matmul(
                            pts[gi][:],
                            wcs[dc][:, 256 + ct * 128:256 + (ct + 1) * 128],
                            xcs[dc][:, th * 512:(th + 1) * 512],
                            start=(dc == 0), stop=(dc == 7),
                        )
                for gi, (ct, th) in enumerate(
                        [(2 * wave + c, t) for c in range(2)
                         for t in range(2)]):
                    nc.scalar.activation(
                        qk[:, ct, th * 512:(th + 1) * 512], pts[gi][:],
                        SILU, bias=PKs[:, PK_BQK + ct:PK_BQK + ct + 1],
                        scale=1.0,
                    )

            # ---- in_proj U packed [128, 2 groups, t] ----
            U128 = big.tile([128, 2, 1024], BF16)
            for g in range(2):
                for th in range(2):
                    pt = ps.tile([128, 512], F32, tag="mm")
                    for dc in range(8):
                        nc.tensor.matmul(
                            pt[:],
                            wcs[dc][:, g * 128:(g + 1) * 128],
                            xcs[dc][:, th * 512:(th + 1) * 512],
                            start=(dc == 0), stop=(dc == 7),
                        )
                    nc.scalar.activation(
                        U128[:, g, th * 512:(th + 1) * 512], pt[:],
                        SILU, bias=PKs[:, PK_BU + g:PK_BU + g + 1], scale=1.0,
                    )

            # ---- beta*U (pre-gamma) if needed, then gU = gamma*U ----
            if with_c:
                bU = scratch.tile([128, 2, 1024], BF16, tag="bu")
                for g in range(2):
                    nc.vector.tensor_scalar_mul(
                        bU[:, g, :], U128[:, g, :],
                        PKs[:, PK_BET + g:PK_BET + g + 1])
            gU = scratch.tile([128, 2, 1024], BF16, tag="gu")
            for g in range(2):
                nc.vector.tensor_scalar_mul(
                    gU[:, g, :], U128[:, g, :],
                    PKs[:, PK_GAM + g:PK_GAM + g + 1])

            # ---- out_proj emitters; interleaved into the attention loops so
            # the in-order PE always has independent matmuls queued while a
            # tile's mask+silu chain is in flight (keeps HAM at full clock)
            def make_bgroup(dst, rhs, dt_, nm):
                def emit():
                    stb = opool.tile([128, 1024], BF16, tag="stB",
                                     name=f"stb_{nm}_{dt_}")
                    for th in range(2):
                        pt = ps.tile([128, 512], F32, tag="mm",
                                     name=f"bpt_{nm}_{dt_}_{th}")
                        for g in range(2):
                            nc.tensor.matmul(
                                pt[:],
                                WOs[:, g, dt_ * 128:(dt_ + 1) * 128],
                                rhs[:, g, th * 512:(th + 1) * 512],
                                start=(g == 0), stop=(g == 1),
                            )
                        nc.vector.tensor_copy(
                            out=stb[:, th * 512:(th + 1) * 512], in_=pt[:])
                    nc.sync.dma_start(dst[dt_], stb[:])
                return emit

            def make_agroup(nUA, dt_, rt):
                half = slice(rt * 512, (rt + 1) * 512)

                def emit():
                    pt = ps.tile([128, 512], F32, tag="mm",
                                 name=f"apt{rt}_{dt_}")
                    for g in range(2):
                        nc.tensor.matmul(
                            pt[:],
                            WOs[:, g, dt_ * 128:(dt_ + 1) * 128],
                            nUA[:, g, half],
                            start=(g == 0), stop=(g == 1),
                        )
                    st = opool.tile([128, 512], BF16, tag="stA",
                                    name=f"sta{rt}_{dt_}")
                    if dt_ % 2 == 0:
                        nc.scalar.activation(st[:], pt[:], IDENT)
                    else:
                        nc.vector.tensor_copy(out=st[:], in_=pt[:])
                    nc.sync.dma_start(AT[dt_, :, half], st[:])
                return emit

            extra_work = [make_bgroup(BT, gU, dt_, "b") for dt_ in range(8)]
            if with_c:
                extra_work += [make_bgroup(CT, bU, dt_, "c")
                               for dt_ in range(8)]

            # ---- in_proj natural: V = silu(x @ W_V^T + b_V) * vmask ----
            # V3 [t%128, t//128, pair, 3 blocks, 64]: head 2g in block 0,
            # head 2g+1 in block 2, block 1 stays zero.  AV lhsT for head j
            # is blocks (j%2) : (j%2)+2 -> 128 free cols, half zeros, so the
            # psum output lands at full partitions with head j's channels at
            # partition base (j%2)*64.
            V3 = big.tile([128, 8, 2, 3, 64], BF16)
            nc.gpsimd.memset(V3[:], 0.0)
            for tt in range(8):
                pt = ps.tile([128, 512], F32, tag="mm")
                for dc in range(8):
                    nc.tensor.matmul(
                        pt[:, :256],
                        xcs[dc][:, tt * 128:(tt + 1) * 128],
                        wv2[dc // 4][:, dc % 4, :],
                        start=(dc == 0), stop=(dc == 7),
                    )
                vs = apool.tile([128, 256], F32, tag="vs")
                nc.vector.tensor_add(vs[:], pt[:, :256],
                                     PKs[:, PK_BVB:PK_BVB + 256])
                for g in range(2):
                    nc.scalar.activation(
                        V3[:, tt, g, 0:3:2, :],
                        vs[:, g * 128:(g + 1) * 128], SILU,
                        scale=PKs[:, PK_VM + tt:PK_VM + tt + 1])

            # ---- attention: one global software pipeline over every
            # (rt, pair, kt) step.  A single pend queue spans pair and rt
            # boundaries so the PE never drains; each head-pair shares one
            # psum accumulator (even head starts the bank, odd head
            # accumulates onto it -- its unused partition half adds zeros).
            AVs = big.tile([128, 2, 1024], F32R)  # [c%128, c//128, t]
            nUA = scratch.tile([128, 2, 1024], BF16, tag="nua")
            sstage = scratch.tile([1, 4, 512], F32, tag="sst")
            sqhs = []
            pend = []
            done_in_rt = [0, 0]

            def rt_epilogue(rt_):
                # nUA per g on DVE, sq on Pool; A-proj groups queued as
                # extra_work so they drain inside the remaining attention
                half = slice(rt_ * 512, (rt_ + 1) * 512)
                sqh = scratch.tile([128, 2, 512], F32R, tag=f"sqh{rt_}",
                                   name=f"sqh{rt_}")
                sqhs.append(sqh)
                nc.gpsimd.tensor_mul(sqh[:], AVs[:, :, half],
                                     AVs[:, :, half])
                for g in range(2):
                    nc.vector.tensor_mul(nUA[:, g, half], AVs[:, g, half],
                                         gU[:, g, half])
                extra_work.extend(
                    make_agroup(nUA, dt_, rt_) for dt_ in range(8))

            def flush_one():
                j_, rt_, att_, off_, kt_, avp_, nkt_ = pend.pop(0)
                nc.tensor.matmul(
                    avp_[:, off_:512],
                    V3[:, kt_, j_ // 2, (j_ % 2):(j_ % 2) + 2, :],
                    att_[:, off_:512],
                    start=(kt_ == 0), stop=(kt_ == nkt_ - 1),
                    skip_group_check=True,
                )
                if kt_ == nkt_ - 1:
                    # head complete: scale by invd into AVs
                    pb_ = (j_ % 2) * 64
                    nc.vector.tensor_mul(
                        AVs[pb_:pb_ + 64, j_ // 2,
                            rt_ * 512:(rt_ + 1) * 512],
                        avp_[pb_:pb_ + 64, :],
                        PKs[pb_:pb_ + 64,
                            PK_INVD + rt_ * 512:PK_INVD + (rt_ + 1) * 512])
                    done_in_rt[rt_] += 1
                    if done_in_rt[rt_] == 4:
                        rt_epilogue(rt_)

            for rt in range(2):
                n_kt = 4 * rt + 4
                for jp in (0, 2):
                    avp_d = {
                        j: psav.tile([128, 512], F32, tag="av",
                                     name=f"avp{rt}_{j}")
                        for j in (jp, jp + 1)
                    }
                    for kt in range(n_kt):
                        d0 = rt * 512 - kt * 128
                        off = max(0, -d0)   # causal-trim: i >= kt*128
                        cs = max(0, d0)
                        for j in (jp, jp + 1):
                            pb = (j % 2) * 64
                            ch = j // 2
                            qkp = ps.tile([128, 512], F32, tag="mm",
                                          name=f"qkp{rt}_{j}_{kt}")
                            nc.tensor.matmul(
                                qkp[:, off:512],
                                qk[pb:pb + 64, 2 + ch,
                                   kt * 128:(kt + 1) * 128],
                                qk[pb:pb + 64, ch,
                                   rt * 512 + off:(rt + 1) * 512],
                                start=True, stop=True,
                            )
                            # alpha*logits + mask window -> SBUF
                            asum = apool.tile([128, 512], F32, tag="asum")
                            nc.vector.scalar_tensor_tensor(
                                asum[:, off:512], qkp[:, off:512], alpha,
                                MTs[:, j, cs:cs + 512 - off], MULT, ADD)
                            att = apool.tile([128, 512], BF16, tag="att")
                            nc.scalar.activation(att[:, off:512],
                                                 asum[:, off:512], SILU)
                            pend.append((j, rt, att, off, kt, avp_d[j],
                                         n_kt))
                            if len(pend) > 4:
                                flush_one()
                            if extra_work:
                                extra_work.pop(0)()
            while pend:
                flush_one()

            while extra_work:
                extra_work.pop(0)()

            # stats (tiny PE work, after all projection matmuls)
            for rt in range(2):
                half = slice(rt * 512, (rt + 1) * 512)
                for si in range(2):
                    sp = psst.tile([1, 512], F32, tag="st")
                    for g in range(2):
                        rhs = (AVs[:, g, half] if si == 0
                               else sqhs[rt][:, g, :])
                        nc.tensor.matmul(
                            sp[:], ONESs[:], rhs,
                            start=(g == 0), stop=(g == 1),
                        )
                    nc.vector.tensor_copy(
                        out=sstage[:, si * 2 + rt, :], in_=sp[:])

            nc.sync.dma_start(SOUT[:], sstage[:])

    nc.compile()
    return nc


_NC_CACHE = {}


def _prep_in_maps(inputs):
    x = np.asarray(inputs["x"], np.float32)
    key_padding_mask = np.asarray(inputs["key_padding_mask"])
    W_in = np.asarray(inputs["W_in"], np.float32)
    b_in = np.asarray(inputs["b_in"], np.float32)
    W_out = np.asarray(inputs["W_out"], np.float32)
    gamma = np.asarray(inputs["gamma"], np.float32)
    beta = np.asarray(inputs["beta"], np.float32)
    rab_emb = np.asarray(inputs["rab_emb"], np.float32)

    lengths = (~key_padding_mask).sum(axis=1)  # valid keys per batch
    in_maps = []
    for core in range(8):
        b, hg = core // 4, core % 4
        sl = slice(hg * 256, hg * 256 + 256)
        Wu = W_in[0:1024][sl]
        Wv = W_in[1024:2048][sl]
        Wq = W_in[2048:3072][sl]
        Wk = W_in[3072:4096][sl]
        WC_np = np.concatenate([Wu, Wq, Wk], 0).T.reshape(8, 128, 768)
        WVC_np = Wv.T.reshape(2, 4, 128, 256).transpose(0, 2, 1, 3)
        XC_np = x[b].T.reshape(8, 128, 1024)
        WO_np = np.ascontiguousarray(
            W_out[:, sl].T.reshape(2, 128, 1024).transpose(1, 0, 2))
        L = int(lengths[b])
        denom = np.clip(np.minimum(np.arange(T) + 1, L), 1, None)
        heads = [4 * hg + jj for jj in range(H_PER)]
        MT_np = _toeplitz_mask_np(rab_emb, heads)

        PK_np = np.zeros((128, PK_N), np.float32)
        bqk = np.concatenate([b_in[2048:3072][sl], b_in[3072:4096][sl]])
        PK_np[:, PK_BQK:PK_BQK + 4] = bqk.reshape(4, 128).T
        PK_np[:, PK_BU:PK_BU + 2] = b_in[0:1024][sl].reshape(2, 128).T
        PK_np[:, PK_GAM:PK_GAM + 2] = gamma[sl].reshape(2, 128).T
        PK_np[:, PK_BET:PK_BET + 2] = beta[sl].reshape(2, 128).T
        PK_np[:, PK_VM:PK_VM + 8] = (
            np.arange(128)[:, None] + 128 * np.arange(8)[None, :] < L)
        PK_np[:, PK_INVD:PK_INVD + 1024] = (1.0 / denom)[None, :]
        PK_np[:, PK_BVB:PK_BVB + 256] = b_in[1024:2048][sl][None, :]

        in_maps.append({
            "XC": np.ascontiguousarray(XC_np).astype(BF16_NP),
            "WC": np.ascontiguousarray(WC_np).astype(BF16_NP),
            "WVC": np.ascontiguousarray(WVC_np).astype(BF16_NP),
            "WO": WO_np.astype(BF16_NP),
            "MT": MT_np.astype(BF16_NP),
            "PK": PK_np,
            "ONESP": np.ones((128, 1), np.float32),
        })
    return in_maps


def kernel(x, attention_mask, key_padding_mask, W_in, b_in, W_out, b_out,
           gamma, beta, rab_emb):
    global LAST_RESULTS
    x = np.asarray(x, np.float32)
    key_padding_mask = np.asarray(key_padding_mask)
    b_out = np.asarray(b_out, np.float32)
    beta = np.asarray(beta, np.float32)

    with_c = bool(np.any(beta != 0.0))
    if with_c not in _NC_CACHE:
        _NC_CACHE[with_c] = _build(with_c)
    nc = _NC_CACHE[with_c]

    in_maps = _prep_in_maps(dict(
        x=x, attention_mask=attention_mask, key_padding_mask=key_padding_mask,
        W_in=W_in, b_in=b_in, W_out=W_out, b_out=b_out, gamma=gamma,
        beta=beta, rab_emb=rab_emb))

    res = run_bass_kernel_spmd(nc, in_maps, list(range(8)))
    LAST_RESULTS = res

    out = np.empty((B, T, D), np.float32)
    for b in range(B):
        A = np.zeros((T, D), np.float64)
        Bm = np.zeros((T, D), np.float64)
        Cm = np.zeros((T, D), np.float64)
        s1 = np.zeros(T, np.float64)
        s2 = np.zeros(T, np.float64)
        for hg in range(4):
            r = res.results[4 * b + hg]
            A += r["AT"].reshape(1024, 1024).T.astype(np.float64)
            Bm += r["BT"].reshape(1024, 1024).T.astype(np.float64)
            if with_c:
                Cm += r["CT"].reshape(1024, 1024).T.astype(np.float64)
            s = r["SOUT"].reshape(4, 512)
            s1 += np.concatenate([s[0], s[1]]).astype(np.float64)
            s2 += np.concatenate([s[2], s[3]]).astype(np.float64)
        # s1, s2 already invd-scaled on device
        mu = s1 / D
        var = s2 / D - mu * mu
        rho = 1.0 / np.sqrt(var + LN_EPS)
        y = (rho[:, None] * A - (rho * mu)[:, None] * Bm + Cm
             + b_out[None, :].astype(np.float64) + x[b].astype(np.float64))
        out[b] = y.astype(np.float32)
    return out


# revision 29
# speedup vs baseline: 1.1903x; 1.0477x over previous
"""HSTU layer kernel for Trainium2, 8 NeuronCores.

Sharding: core = 4*b + hg  (b in {0,1} data-parallel over batch,
hg in {0..3} head-parallel: 4 heads = 256 channels of U/V/Q/K each).

v6 @112us (baseline 220us). Key techniques, in order of impact:
  - bf16 operands everywhere (tolerance 2e-2; psum/stats stay f32),
    full 128-partition contraction/output packing on every matmul,
    causal-trimmed matmul widths in logits/AV
  - mask is Toeplitz in (i-j): resident [128, 4, 1024] window table
    replaces the 16MB streamed dense mask; key padding handled by
    zeroing V rows (silu scale=0); alpha folded into the mask-add
  - PE kept dense so the HAM clock gate stays at 2.4GHz: head-pair
    interleaved, depth-4 software-pipelined attention; out_proj matmul
    groups popped from a work queue inside the attention loops; Q-wave
    in_proj runs dc-outer so it pipelines with the input DMA stream
  - V stored in a 3-block [head_even | zeros | head_odd] layout so the
    AV matmul lhsT spans 128 free cols (psum output at full partitions,
    base 0 -- a psum write at column offset 64 crashes the HW)
  - DMA issue split across both HWDGE engines (sync + scalar), small
    inputs packed into one [128, 1298] tensor, bf16 outputs
  - stats/nUA elementwise work spread over DVE + Pool

Per core (channels-on-partitions, zero device transposes):
  qk^T = silu(W_qk @ x^T + b)            [128, 4 ct, t] bf16
  U    = silu(W_u @ x^T + b)             [128, 2 g, t] bf16; gU = gamma*U
  V    = silu(x @ W_v^T + b_v) * vmask   3-block layout, bf16
  logits^T[j,i] = K_h @ Q_h^T            (psum f32)
  att^T = silu(alpha*logits^T + MT_win)  bf16   (MT Toeplitz window table)
  AVs^T_h = (V_h^T @ att^T) * invd       [128, 2 g, t] f32r
  s1 = sum_c AVs, s2 = sum_c AVs^2       (ones-matmul, contraction 128)
  A^T = WO^T @ (AVs*gU)^T ; B^T = WO^T @ gU^T   (out_proj partials)

Host combine (LayerNorm is linear in its input given row stats):
  y = rho*A - (rho*mu)*B (+ C) + b_out + x
"""
import math
import numpy as np
import ml_dtypes

import concourse.bass as bass
import concourse.mybir as mybir
import concourse.tile as tile
from concourse import bacc
from concourse.bass_utils import run_bass_kernel_spmd

NUM_HEADS = 16
NUM_BUCKETS = 32
MAX_DISTANCE = 128
NEG_INF = -1e9
LN_EPS = 1e-5

B, T, D = 2, 1024, 1024
H_PER = 4           # heads per core
F32R = mybir.dt.float32r
F32 = mybir.dt.float32
BF16 = mybir.dt.bfloat16
BF16_NP = ml_dtypes.bfloat16

# packed-small-input column offsets
PK_BQK, PK_BU, PK_GAM, PK_BET = 0, 4, 6, 8
PK_VM, PK_INVD, PK_BVB, PK_N = 10, 18, 1042, 1298

LAST_RESULTS = None


def _bucket_np(n):
    """T5-style log bucket for clamped distance n >= 0."""
    max_exact = NUM_BUCKETS // 2
    with np.errstate(divide="ignore", invalid="ignore"):
        large = max_exact + (
            np.log(n.astype(np.float32) / max_exact + 1e-6)
            / math.log(MAX_DISTANCE / max_exact)
            * (NUM_BUCKETS - max_exact)
        ).astype(np.int32)
    large = np.minimum(large, NUM_BUCKETS - 1)
    return np.where(n < max_exact, n, large)


def _toeplitz_mask_np(rab_emb, heads):
    """MT[p, jj, c] = rab_h(c - p) for c-p >= 0 else -1e9.  [128, 4, 1024]"""
    d = np.arange(T)[None, :] - np.arange(128)[:, None]   # [128, 1024] = c - p
    n = np.clip(d, 0, None)
    buckets = _bucket_np(n)                               # [128, 1024]
    out = np.empty((128, len(heads), T), np.float32)
    for jj, h in enumerate(heads):
        out[:, jj, :] = np.where(d < 0, NEG_INF, rab_emb[buckets, h])
    return np.ascontiguousarray(out)


def _build(with_c):
    nc = bacc.Bacc("TRN2", target_bir_lowering=False, debug=False, num_devices=8)

    def inp(name, shape, dt):
        return nc.dram_tensor(name, shape, dt, kind="ExternalInput").ap()

    XC = inp("XC", [8, 128, 1024], BF16)     # x[b].T d-chunks
    WC = inp("WC", [8, 128, 768], BF16)      # W_in.T d-chunks: [U(256)|Q(256)|K(256)]
    WVC = inp("WVC", [2, 128, 4, 256], BF16)  # W_v.T d-chunks, 2 bundles
    WO = inp("WO", [128, 2, 1024], BF16)     # W_out cols slice: [c%128, c//128, dout]
    MT = inp("MT", [128, 4, 1024], BF16)     # Toeplitz mask windows per local head
    PK = inp("PK", [128, PK_N], F32)         # packed biases/gamma/beta/vmask/invd/bvb
    ONESP = inp("ONESP", [128, 1], F32R)

    AT = nc.dram_tensor("AT", [8, 128, 1024], BF16, kind="ExternalOutput").ap()
    BT = nc.dram_tensor("BT", [8, 128, 1024], BF16, kind="ExternalOutput").ap()
    CT = (nc.dram_tensor("CT", [8, 128, 1024], BF16, kind="ExternalOutput").ap()
          if with_c else None)
    SOUT = nc.dram_tensor("SOUT", [1, 4, 512], F32, kind="ExternalOutput").ap()

    alpha = (D // NUM_HEADS) ** (-0.5)
    SILU = mybir.ActivationFunctionType.Silu
    IDENT = mybir.ActivationFunctionType.Identity
    MULT = mybir.AluOpType.mult
    ADD = mybir.AluOpType.add

    with tile.TileContext(nc) as tc:
        with (
            tc.tile_pool(name="big", bufs=1) as big,
            tc.tile_pool(name="scratch", bufs=1) as scratch,
            tc.tile_pool(name="apool", bufs=8) as apool,
            tc.tile_pool(name="opool", bufs=3) as opool,
            tc.tile_pool(name="ps", bufs=6, space="PSUM") as ps,
            tc.tile_pool(name="psav", bufs=2, space="PSUM") as psav,
        ):
            # ---- resident loads: x chunks on sync, W chunks on scalar so
            # issue serialization doesn't gate the in_proj start ----
            xcs, wcs = [], []
            for dc in range(4):
                xt = big.tile([128, 1024], BF16, tag=f"xc{dc}")
                nc.sync.dma_start(xt[:], XC[dc])
                xcs.append(xt)
            PKs = big.tile([128, PK_N], F32)
            nc.sync.dma_start(PKs[:], PK[:])
            for dc in range(4, 8):
                xt = big.tile([128, 1024], BF16, tag=f"xc{dc}")
                nc.sync.dma_start(xt[:], XC[dc])
                xcs.append(xt)
            for dc in range(8):
                wt = big.tile([128, 768], BF16, tag=f"wc{dc}")
                nc.scalar.dma_start(wt[:], WC[dc])
                wcs.append(wt)
            wv2 = []
            for bi in range(2):
                wv = big.tile([128, 4, 256], BF16, tag=f"wv{bi}")
                (nc.sync if bi == 0 else nc.scalar).dma_start(wv[:], WVC[bi])
                wv2.append(wv)
            WOs = big.tile([128, 2, 1024], BF16)
            nc.scalar.dma_start(WOs[:], WO[:])
            MTs = big.tile([128, 4, 1024], BF16)
            nc.sync.dma_start(MTs[:], MT[:])
            ONESs = big.tile([128, 1], F32R)
            nc.scalar.dma_start(ONESs[:], ONESP[:])

            # ---- in_proj Q,K transposed: qk = silu(W @ x^T + b) bf16 ----
            # Q wave runs dc-OUTER over 4 live psum groups: each x/W chunk is
            # consumed for all groups as soon as its DMA lands, so the PE
            # pipelines with the input stream instead of stalling on chunk 7.
            qk = big.tile([128, 4, 1024], BF16)  # ct 0,1=Q  2,3=K
            for wave in range(2):                # wave 0 = Q (ct 0,1), 1 = K
                pts = [ps.tile([128, 512], F32, tag="mm", name=f"ptw{wave}_{i}")
                       for i in range(4)]
                for dc in range(8):
                    for gi, (ct, th) in enumerate(
                            [(2 * wave + c, t) for c in range(2)
                             for t in range(2)]):
                        nc.tensor.matmul(
                            pts[gi][:],
                            wcs[dc][:, 256 + ct * 128:256 + (ct + 1) * 128],
                            xcs[dc][:, th * 512:(th + 1) * 512],
                            start=(dc == 0), stop=(dc == 7),
                        )
                for gi, (ct, th) in enumerate(
                        [(2 * wave + c, t) for c in range(2)
                         for t in range(2)]):
                    nc.scalar.activation(
                        qk[:, ct, th * 512:(th + 1) * 512], pts[gi][:],
                        SILU, bias=PKs[:, PK_BQK + ct:PK_BQK + ct + 1],
                        scale=1.0,
                    )

            # ---- in_proj U packed [128, 2 groups, t] ----
            U128 = big.tile([128, 2, 1024], BF16)
            for g in range(2):
                for th in range(2):
                    pt = ps.tile([128, 512], F32, tag="mm")
                    for dc in range(8):
                        nc.tensor.matmul(
                            pt[:],
                            wcs[dc][:, g * 128:(g + 1) * 128],
                            xcs[dc][:, th * 512:(th + 1) * 512],
                            start=(dc == 0), stop=(dc == 7),
                        )
                    nc.scalar.activation(
                        U128[:, g, th * 512:(th + 1) * 512], pt[:],
                        SILU, bias=PKs[:, PK_BU + g:PK_BU + g + 1], scale=1.0,
                    )

            # ---- beta*U (pre-gamma) if needed, then gU = gamma*U ----
            if with_c:
                bU = scratch.tile([128, 2, 1024], BF16, tag="bu")
                for g in range(2):
                    nc.vector.tensor_scalar_mul(
                        bU[:, g, :], U128[:, g, :],
                        PKs[:, PK_BET + g:PK_BET + g + 1])
            gU = scratch.tile([128, 2, 1024], BF16, tag="gu")
            for g in range(2):
                nc.vector.tensor_scalar_mul(
                    gU[:, g, :], U128[:, g, :],
                    PKs[:, PK_GAM + g:PK_GAM + g + 1])

            # ---- out_proj emitters; interleaved into the attention loops so
            # the in-order PE always has independent matmuls queued while a
            # tile's mask+silu chain is in flight (keeps HAM at full clock)
            def make_bgroup(dst, rhs, dt_, nm):
                def emit():
                    stb = opool.tile([128, 1024], BF16, tag="stB",
                                     name=f"stb_{nm}_{dt_}")
                    for th in range(2):
                        pt = ps.tile([128, 512], F32, tag="mm",
                                     name=f"bpt_{nm}_{dt_}_{th}")
                        for g in range(2):
                            nc.tensor.matmul(
                                pt[:],
                                WOs[:, g, dt_ * 128:(dt_ + 1) * 128],
                                rhs[:, g, th * 512:(th + 1) * 512],
                                start=(g == 0), stop=(g == 1),
                            )
                        nc.vector.tensor_copy(
                            out=stb[:, th * 512:(th + 1) * 512], in_=pt[:])
                    nc.sync.dma_start(dst[dt_], stb[:])
                return emit

            def make_agroup(nUA, dt_, rt):
                half = slice(rt * 512, (rt + 1) * 512)

                def emit():
                    pt = ps.tile([128, 512], F32, tag="mm",
                                 name=f"apt{rt}_{dt_}")
                    for g in range(2):
                        nc.tensor.matmul(
                            pt[:],
                            WOs[:, g, dt_ * 128:(dt_ + 1) * 128],
                            nUA[:, g, half],
                            start=(g == 0), stop=(g == 1),
                        )
                    st = opool.tile([128, 512], BF16, tag="stA",
                                    name=f"sta{rt}_{dt_}")
                    if dt_ % 2 == 0:
                        nc.scalar.activation(st[:], pt[:], IDENT)
                    else:
                        nc.vector.tensor_copy(out=st[:], in_=pt[:])
                    (nc.sync if dt_ % 2 == 0 else nc.scalar).dma_start(
                        AT[dt_, :, half], st[:])
                return emit

            extra_work = [make_bgroup(BT, gU, dt_, "b") for dt_ in range(8)]
            if with_c:
                extra_work += [make_bgroup(CT, bU, dt_, "c")
                               for dt_ in range(8)]

            # ---- in_proj natural: V = silu(x @ W_V^T + b_V) * vmask ----
            # V3 [t%128, t//128, pair, 3 blocks, 64]: head 2g in block 0,
            # head 2g+1 in block 2, block 1 stays zero.  AV lhsT for head j
            # is blocks (j%2) : (j%2)+2 -> 128 free cols, half zeros, so the
            # psum output lands at full partitions with head j's channels at
            # partition base (j%2)*64.
            V3 = big.tile([128, 8, 2, 3, 64], BF16)
            nc.gpsimd.memset(V3[:], 0.0)
            for tt in range(8):
                pt = ps.tile([128, 512], F32, tag="mm")
                for dc in range(8):
                    nc.tensor.matmul(
                        pt[:, :256],
                        xcs[dc][:, tt * 128:(tt + 1) * 128],
                        wv2[dc // 4][:, dc % 4, :],
                        start=(dc == 0), stop=(dc == 7),
                    )
                vs = apool.tile([128, 256], F32, tag="vs")
                nc.vector.tensor_add(vs[:], pt[:, :256],
                                     PKs[:, PK_BVB:PK_BVB + 256])
                for g in range(2):
                    nc.scalar.activation(
                        V3[:, tt, g, 0:3:2, :],
                        vs[:, g * 128:(g + 1) * 128], SILU,
                        scale=PKs[:, PK_VM + tt:PK_VM + tt + 1])

            # ---- attention, rt-major; A-proj for each half interleaved ----
            AVs = big.tile([128, 2, 1024], F32R)  # [c%128, c//128, t]
            nUA = scratch.tile([128, 2, 1024], BF16, tag="nua")
            sstage = scratch.tile([1, 4, 512], F32, tag="sst")
            sqhs = []
            for rt in range(2):
                n_kt = 4 * rt + 4
                # head pairs interleaved: two independent mask+silu chains
                # feed the PE so it never outruns a single chain's latency
                for jp in (0, 2):
                    avp_d = {
                        j: psav.tile([128, 512], F32, tag="av",
                                     name=f"avp{rt}_{j}")
                        for j in (jp, jp + 1)
                    }
                    pend = []

                    def flush_one():
                        j_, att_, off_, kt_ = pend.pop(0)
                        nc.tensor.matmul(
                            avp_d[j_][:, off_:512],
                            V3[:, kt_, j_ // 2, (j_ % 2):(j_ % 2) + 2, :],
                            att_[:, off_:512],
                            start=(kt_ == 0), stop=(kt_ == n_kt - 1),
                            skip_group_check=True,
                        )

                    for kt in range(n_kt):
                        d0 = rt * 512 - kt * 128
                        off = max(0, -d0)   # causal-trim: i >= kt*128
                        cs = max(0, d0)
                        for j in (jp, jp + 1):
                            pb = (j % 2) * 64
                            ch = j // 2
                            qkp = ps.tile([128, 512], F32, tag="mm",
                                          name=f"qkp{rt}_{j}_{kt}")
                            nc.tensor.matmul(
                                qkp[:, off:512],
                                qk[pb:pb + 64, 2 + ch,
                                   kt * 128:(kt + 1) * 128],
                                qk[pb:pb + 64, ch,
                                   rt * 512 + off:(rt + 1) * 512],
                                start=True, stop=True,
                            )
                            # alpha*logits + mask window -> SBUF
                            asum = apool.tile([128, 512], F32, tag="asum")
                            nc.vector.scalar_tensor_tensor(
                                asum[:, off:512], qkp[:, off:512], alpha,
                                MTs[:, j, cs:cs + 512 - off], MULT, ADD)
                            att = apool.tile([128, 512], BF16, tag="att")
                            nc.scalar.activation(att[:, off:512],
                                                 asum[:, off:512], SILU)
                            pend.append((j, att, off, kt))
                            if len(pend) > 5:
                                flush_one()
                            if extra_work:
                                extra_work.pop(0)()
                    while pend:
                        flush_one()
                    for j in (jp, jp + 1):
                        pb = (j % 2) * 64
                        nc.vector.tensor_mul(
                            AVs[pb:pb + 64, j // 2, rt * 512:(rt + 1) * 512],
                            avp_d[j][pb:pb + 64, :],
                            PKs[pb:pb + 64,
                                PK_INVD + rt * 512:PK_INVD + (rt + 1) * 512])

                # nUA per g on DVE, sq on Pool in parallel; A-proj groups for
                # this half are queued as extra_work (rt0 drains inside rt1's
                # attention loop; rt1's leftovers drain below)
                half = slice(rt * 512, (rt + 1) * 512)
                sqh = scratch.tile([128, 2, 512], F32R, tag=f"sqh{rt}",
                                   name=f"sqh{rt}")
                sqhs.append(sqh)
                nc.gpsimd.tensor_mul(sqh[:], AVs[:, :, half], AVs[:, :, half])
                for g in range(2):
                    nc.vector.tensor_mul(nUA[:, g, half], AVs[:, g, half],
                                         gU[:, g, half])
                extra_work.extend(
                    make_agroup(nUA, dt_, rt) for dt_ in range(8))

            while extra_work:
                extra_work.pop(0)()

            # stats (tiny PE work, after all projection matmuls)
            for rt in range(2):
                half = slice(rt * 512, (rt + 1) * 512)
                for si in range(2):
                    sp = ps.tile([128, 512], F32, tag="mm",
                                 name=f"sp{rt}_{si}")
                    for g in range(2):
                        rhs = (AVs[:, g, half] if si == 0
                               else sqhs[rt][:, g, :])
                        nc.tensor.matmul(
                            sp[0:1, :], ONESs[:], rhs,
                            start=(g == 0), stop=(g == 1),
                        )
                    nc.vector.tensor_copy(
                        out=sstage[:, si * 2 + rt, :], in_=sp[0:1, :])

            nc.scalar.dma_start(SOUT[:], sstage[:])

    nc.compile()
    return nc


_NC_CACHE = {}


def _prep_in_maps(inputs):
    x = np.asarray(inputs["x"], np.float32)
    key_padding_mask = np.asarray(inputs["key_padding_mask"])
    W_in = np.asarray(inputs["W_in"], np.float32)
    b_in = np.asarray(inputs["b_in"], np.float32)
    W_out = np.asarray(inputs["W_out"], np.float32)
    gamma = np.asarray(inputs["gamma"], np.float32)
    beta = np.asarray(inputs["beta"], np.float32)
    rab_emb = np.asarray(inputs["rab_emb"], np.float32)

    lengths = (~key_padding_mask).sum(axis=1)  # valid keys per batch
    in_maps = []
    for core in range(8):
        b, hg = core // 4, core % 4
        sl = slice(hg * 256, hg * 256 + 256)
        Wu = W_in[0:1024][sl]
        Wv = W_in[1024:2048][sl]
        Wq = W_in[2048:3072][sl]
        Wk = W_in[3072:4096][sl]
        WC_np = np.concatenate([Wu, Wq, Wk], 0).T.reshape(8, 128, 768)
        WVC_np = Wv.T.reshape(2, 4, 128, 256).transpose(0, 2, 1, 3)
        XC_np = x[b].T.reshape(8, 128, 1024)
        WO_np = np.ascontiguousarray(
            W_out[:, sl].T.reshape(2, 128, 1024).transpose(1, 0, 2))
        L = int(lengths[b])
        denom = np.clip(np.minimum(np.arange(T) + 1, L), 1, None)
        heads = [4 * hg + jj for jj in range(H_PER)]
        MT_np = _toeplitz_mask_np(rab_emb, heads)

        PK_np = np.zeros((128, PK_N), np.float32)
        bqk = np.concatenate([b_in[2048:3072][sl], b_in[3072:4096][sl]])
        PK_np[:, PK_BQK:PK_BQK + 4] = bqk.reshape(4, 128).T
        PK_np[:, PK_BU:PK_BU + 2] = b_in[0:1024][sl].reshape(2, 128).T
        PK_np[:, PK_GAM:PK_GAM + 2] = gamma[sl].reshape(2, 128).T
        PK_np[:, PK_BET:PK_BET + 2] = beta[sl].reshape(2, 128).T
        PK_np[:, PK_VM:PK_VM + 8] = (
            np.arange(128)[:, None] + 128 * np.arange(8)[None, :] < L)
        PK_np[:, PK_INVD:PK_INVD + 1024] = (1.0 / denom)[None, :]
        PK_np[:, PK_BVB:PK_BVB + 256] = b_in[1024:2048][sl][None, :]

        in_maps.append({
            "XC": np.ascontiguousarray(XC_np).astype(BF16_NP),
            "WC": np.ascontiguousarray(WC_np).astype(BF16_NP),
            "WVC": np.ascontiguousarray(WVC_np).astype(BF16_NP),
            "WO": WO_np.astype(BF16_NP),
            "MT": MT_np.astype(BF16_NP),
            "PK": PK_np,
            "ONESP": np.ones((128, 1), np.float32),
        })
    return in_maps


def kernel(x, attention_mask, key_padding_mask, W_in, b_in, W_out, b_out,
           gamma, beta, rab_emb):
    global LAST_RESULTS
    x = np.asarray(x, np.float32)
    key_padding_mask = np.asarray(key_padding_mask)
    b_out = np.asarray(b_out, np.float32)
    beta = np.asarray(beta, np.float32)

    with_c = bool(np.any(beta != 0.0))
    if with_c not in _NC_CACHE:
        _NC_CACHE[with_c] = _build(with_c)
    nc = _NC_CACHE[with_c]

    in_maps = _prep_in_maps(dict(
        x=x, attention_mask=attention_mask, key_padding_mask=key_padding_mask,
        W_in=W_in, b_in=b_in, W_out=W_out, b_out=b_out, gamma=gamma,
        beta=beta, rab_emb=rab_emb))

    res = run_bass_kernel_spmd(nc, in_maps, list(range(8)))
    LAST_RESULTS = res

    out = np.empty((B, T, D), np.float32)
    for b in range(B):
        A = np.zeros((T, D), np.float64)
        Bm = np.zeros((T, D), np.float64)
        Cm = np.zeros((T, D), np.float64)
        s1 = np.zeros(T, np.float64)
        s2 = np.zeros(T, np.float64)
        for hg in range(4):
            r = res.results[4 * b + hg]
            A += r["AT"].reshape(1024, 1024).T.astype(np.float64)
            Bm += r["BT"].reshape(1024, 1024).T.astype(np.float64)
            if with_c:
                Cm += r["CT"].reshape(1024, 1024).T.astype(np.float64)
            s = r["SOUT"].reshape(4, 512)
            s1 += np.concatenate([s[0], s[1]]).astype(np.float64)
            s2 += np.concatenate([s[2], s[3]]).astype(np.float64)
        # s1, s2 already invd-scaled on device
        mu = s1 / D
        var = s2 / D - mu * mu
        rho = 1.0 / np.sqrt(var + LN_EPS)
        y = (rho[:, None] * A - (rho * mu)[:, None] * Bm + Cm
             + b_out[None, :].astype(np.float64) + x[b].astype(np.float64))
        out[b] = y.astype(np.float32)
    return out


# revision 31
# speedup vs baseline: 1.2183x; 1.0235x over previous
"""HSTU layer kernel for Trainium2, 8 NeuronCores.

Sharding: core = 4*b + hg  (b in {0,1} data-parallel over batch,
hg in {0..3} head-parallel: 4 heads = 256 channels of U/V/Q/K each).

v9 @107.5us (baseline 220us). Key techniques, in order of impact:
  - bf16 operands everywhere (tolerance 2e-2; psum/stats stay f32),
    full 128-partition contraction/output packing on every matmul,
    causal-trimmed matmul widths in logits/AV
  - mask is Toeplitz in (i-j): resident [128, 4, 1024] window table
    replaces the 16MB streamed dense mask; key padding handled by
    zeroing V rows (silu scale=0); alpha folded into the mask-add
  - PE kept dense so the HAM clock gate stays at 2.4GHz: head-pair
    interleaved, depth-5 software-pipelined attention (6 psum mm slots;
    stats share the mm pool instead of a dedicated bank); out_proj matmul
    groups popped from a work queue inside the attention loops; Q-wave
    in_proj runs dc-outer so it pipelines with the input DMA stream
  - V stored in a 3-block [head_even | zeros | head_odd] layout so the
    AV matmul lhsT spans 128 free cols (psum output at full partitions,
    base 0 -- a psum write at column offset 64 crashes the HW)
  - DMA issue split across both HWDGE engines (sync + scalar), small
    inputs packed into one [128, 1298] tensor, bf16 outputs
  - stats/nUA elementwise work spread over DVE + Pool

Per core (channels-on-partitions, zero device transposes):
  qk^T = silu(W_qk @ x^T + b)            [128, 4 ct, t] bf16
  U    = silu(W_u @ x^T + b)             [128, 2 g, t] bf16; gU = gamma*U
  V    = silu(x @ W_v^T + b_v) * vmask   3-block layout, bf16
  logits^T[j,i] = K_h @ Q_h^T            (psum f32)
  att^T = silu(alpha*logits^T + MT_win)  bf16   (MT Toeplitz window table)
  AVs^T_h = (V_h^T @ att^T) * invd       [128, 2 g, t] f32r
  s1 = sum_c AVs, s2 = sum_c AVs^2       (ones-matmul, contraction 128)
  A^T = WO^T @ (AVs*gU)^T ; B^T = WO^T @ gU^T   (out_proj partials)

Host combine (LayerNorm is linear in its input given row stats):
  y = rho*A - (rho*mu)*B (+ C) + b_out + x
"""
import math
import numpy as np
import ml_dtypes

import concourse.bass as bass
import concourse.mybir as mybir
import concourse.tile as tile
from concourse import bacc
from concourse.bass_utils import run_bass_kernel_spmd

NUM_HEADS = 16
NUM_BUCKETS = 32
MAX_DISTANCE = 128
NEG_INF = -1e9
LN_EPS = 1e-5

B, T, D = 2, 1024, 1024
H_PER = 4           # heads per core
F32R = mybir.dt.float32r
F32 = mybir.dt.float32
BF16 = mybir.dt.bfloat16
BF16_NP = ml_dtypes.bfloat16

# packed-small-input column offsets
PK_BQK, PK_BU, PK_GAM, PK_BET = 0, 4, 6, 8
PK_VM, PK_INVD, PK_BVB, PK_N = 10, 18, 1042, 1298

LAST_RESULTS = None


def _bucket_np(n):
    """T5-style log bucket for clamped distance n >= 0."""
    max_exact = NUM_BUCKETS // 2
    with np.errstate(divide="ignore", invalid="ignore"):
        large = max_exact + (
            np.log(n.astype(np.float32) / max_exact + 1e-6)
            / math.log(MAX_DISTANCE / max_exact)
            * (NUM_BUCKETS - max_exact)
        ).astype(np.int32)
    large = np.minimum(large, NUM_BUCKETS - 1)
    return np.where(n < max_exact, n, large)


def _toeplitz_mask_np(rab_emb, heads):
    """MT[p, jj, c] = rab_h(c - p) for c-p >= 0 else -1e9.  [128, 4, 1024]"""
    d = np.arange(T)[None, :] - np.arange(128)[:, None]   # [128, 1024] = c - p
    n = np.clip(d, 0, None)
    buckets = _bucket_np(n)                               # [128, 1024]
    out = np.empty((128, len(heads), T), np.float32)
    for jj, h in enumerate(heads):
        out[:, jj, :] = np.where(d < 0, NEG_INF, rab_emb[buckets, h])
    return np.ascontiguousarray(out)


def _build(with_c):
    nc = bacc.Bacc("TRN2", target_bir_lowering=False, debug=False, num_devices=8)

    def inp(name, shape, dt):
        return nc.dram_tensor(name, shape, dt, kind="ExternalInput").ap()

    XC = inp("XC", [8, 128, 1024], BF16)     # x[b].T d-chunks
    WC = inp("WC", [8, 128, 768], BF16)      # W_in.T d-chunks: [U(256)|Q(256)|K(256)]
    WVC = inp("WVC", [2, 128, 4, 256], BF16)  # W_v.T d-chunks, 2 bundles
    WO = inp("WO", [128, 2, 1024], BF16)     # W_out cols slice: [c%128, c//128, dout]
    MT = inp("MT", [128, 4, 1024], BF16)     # Toeplitz mask windows per local head
    PK = inp("PK", [128, PK_N], F32)         # packed biases/gamma/beta/vmask/invd/bvb
    ONESP = inp("ONESP", [128, 1], F32R)

    AT = nc.dram_tensor("AT", [8, 128, 1024], BF16, kind="ExternalOutput").ap()
    BT = nc.dram_tensor("BT", [8, 128, 1024], BF16, kind="ExternalOutput").ap()
    CT = (nc.dram_tensor("CT", [8, 128, 1024], BF16, kind="ExternalOutput").ap()
          if with_c else None)
    SOUT = nc.dram_tensor("SOUT", [1, 4, 512], F32, kind="ExternalOutput").ap()

    alpha = (D // NUM_HEADS) ** (-0.5)
    SILU = mybir.ActivationFunctionType.Silu
    IDENT = mybir.ActivationFunctionType.Identity
    MULT = mybir.AluOpType.mult
    ADD = mybir.AluOpType.add

    with tile.TileContext(nc) as tc:
        with (
            tc.tile_pool(name="big", bufs=1) as big,
            tc.tile_pool(name="scratch", bufs=1) as scratch,
            tc.tile_pool(name="apool", bufs=10) as apool,
            tc.tile_pool(name="opool", bufs=4) as opool,
            tc.tile_pool(name="ps", bufs=6, space="PSUM") as ps,
            tc.tile_pool(name="psav", bufs=2, space="PSUM") as psav,
        ):
            # ---- resident loads: x chunks on sync, W chunks on scalar so
            # issue serialization doesn't gate the in_proj start ----
            xcs, wcs = [], []
            for dc in range(4):
                xt = big.tile([128, 1024], BF16, tag=f"xc{dc}")
                nc.sync.dma_start(xt[:], XC[dc])
                xcs.append(xt)
            PKs = big.tile([128, PK_N], F32)
            nc.sync.dma_start(PKs[:], PK[:])
            for dc in range(4, 8):
                xt = big.tile([128, 1024], BF16, tag=f"xc{dc}")
                nc.sync.dma_start(xt[:], XC[dc])
                xcs.append(xt)
            for dc in range(8):
                wt = big.tile([128, 768], BF16, tag=f"wc{dc}")
                nc.scalar.dma_start(wt[:], WC[dc])
                wcs.append(wt)
            wv2 = []
            for bi in range(2):
                wv = big.tile([128, 4, 256], BF16, tag=f"wv{bi}")
                (nc.sync if bi == 0 else nc.scalar).dma_start(wv[:], WVC[bi])
                wv2.append(wv)
            WOs = big.tile([128, 2, 1024], BF16)
            nc.scalar.dma_start(WOs[:], WO[:])
            MTs = big.tile([128, 4, 1024], BF16)
            nc.sync.dma_start(MTs[:], MT[:])
            ONESs = big.tile([128, 1], F32R)
            nc.scalar.dma_start(ONESs[:], ONESP[:])

            # ---- in_proj Q,K transposed: qk = silu(W @ x^T + b) bf16 ----
            # Q wave runs dc-OUTER over 4 live psum groups: each x/W chunk is
            # consumed for all groups as soon as its DMA lands, so the PE
            # pipelines with the input stream instead of stalling on chunk 7.
            qk = big.tile([128, 4, 1024], BF16)  # ct 0,1=Q  2,3=K
            for wave in range(2):                # wave 0 = Q (ct 0,1), 1 = K
                pts = [ps.tile([128, 512], F32, tag="mm", name=f"ptw{wave}_{i}")
                       for i in range(4)]
                for dc in range(8):
                    for gi, (ct, th) in enumerate(
                            [(2 * wave + c, t) for c in range(2)
                             for t in range(2)]):
                        nc.tensor.matmul(
                            pts[gi][:],
                            wcs[dc][:, 256 + ct * 128:256 + (ct + 1) * 128],
                            xcs[dc][:, th * 512:(th + 1) * 512],
                            start=(dc == 0), stop=(dc == 7),
                        )
                for gi, (ct, th) in enumerate(
                        [(2 * wave + c, t) for c in range(2)
                         for t in range(2)]):
                    nc.scalar.activation(
                        qk[:, ct, th * 512:(th + 1) * 512], pts[gi][:],
                        SILU, bias=PKs[:, PK_BQK + ct:PK_BQK + ct + 1],
                        scale=1.0,
                    )

            # ---- in_proj U packed [128, 2 groups, t] ----
            U128 = big.tile([128, 2, 1024], BF16)
            for g in range(2):
                for th in range(2):
                    pt = ps.tile([128, 512], F32, tag="mm")
                    for dc in range(8):
                        nc.tensor.matmul(
                            pt[:],
                            wcs[dc][:, g * 128:(g + 1) * 128],
                            xcs[dc][:, th * 512:(th + 1) * 512],
                            start=(dc == 0), stop=(dc == 7),
                        )
                    nc.scalar.activation(
                        U128[:, g, th * 512:(th + 1) * 512], pt[:],
                        SILU, bias=PKs[:, PK_BU + g:PK_BU + g + 1], scale=1.0,
                    )

            # ---- beta*U (pre-gamma) if needed, then gU = gamma*U ----
            if with_c:
                bU = scratch.tile([128, 2, 1024], BF16, tag="bu")
                for g in range(2):
                    nc.vector.tensor_scalar_mul(
                        bU[:, g, :], U128[:, g, :],
                        PKs[:, PK_BET + g:PK_BET + g + 1])
            gU = scratch.tile([128, 2, 1024], BF16, tag="gu")
            for g in range(2):
                nc.vector.tensor_scalar_mul(
                    gU[:, g, :], U128[:, g, :],
                    PKs[:, PK_GAM + g:PK_GAM + g + 1])

            # ---- out_proj emitters; interleaved into the attention loops so
            # the in-order PE always has independent matmuls queued while a
            # tile's mask+silu chain is in flight (keeps HAM at full clock)
            def make_bgroup(dst, rhs, dt_, nm):
                def emit():
                    stb = opool.tile([128, 1024], BF16, tag="stB",
                                     name=f"stb_{nm}_{dt_}")
                    for th in range(2):
                        pt = ps.tile([128, 512], F32, tag="mm",
                                     name=f"bpt_{nm}_{dt_}_{th}")
                        for g in range(2):
                            nc.tensor.matmul(
                                pt[:],
                                WOs[:, g, dt_ * 128:(dt_ + 1) * 128],
                                rhs[:, g, th * 512:(th + 1) * 512],
                                start=(g == 0), stop=(g == 1),
                            )
                        nc.vector.tensor_copy(
                            out=stb[:, th * 512:(th + 1) * 512], in_=pt[:])
                    nc.sync.dma_start(dst[dt_], stb[:])
                return emit

            def make_agroup(nUA, dt_, rt):
                half = slice(rt * 512, (rt + 1) * 512)

                def emit():
                    pt = ps.tile([128, 512], F32, tag="mm",
                                 name=f"apt{rt}_{dt_}")
                    for g in range(2):
                        nc.tensor.matmul(
                            pt[:],
                            WOs[:, g, dt_ * 128:(dt_ + 1) * 128],
                            nUA[:, g, half],
                            start=(g == 0), stop=(g == 1),
                        )
                    st = opool.tile([128, 512], BF16, tag="stA",
                                    name=f"sta{rt}_{dt_}")
                    if dt_ % 2 == 0:
                        nc.scalar.activation(st[:], pt[:], IDENT)
                    else:
                        nc.vector.tensor_copy(out=st[:], in_=pt[:])
                    (nc.sync if dt_ % 2 == 0 else nc.scalar).dma_start(
                        AT[dt_, :, half], st[:])
                return emit

            extra_work = [make_bgroup(BT, gU, dt_, "b") for dt_ in range(8)]
            if with_c:
                extra_work += [make_bgroup(CT, bU, dt_, "c")
                               for dt_ in range(8)]

            # ---- in_proj natural: V = silu(x @ W_V^T + b_V) * vmask ----
            # V3 [t%128, t//128, pair, 3 blocks, 64]: head 2g in block 0,
            # head 2g+1 in block 2, block 1 stays zero.  AV lhsT for head j
            # is blocks (j%2) : (j%2)+2 -> 128 free cols, half zeros, so the
            # psum output lands at full partitions with head j's channels at
            # partition base (j%2)*64.
            V3 = big.tile([128, 8, 2, 3, 64], BF16)
            nc.gpsimd.memset(V3[:], 0.0)
            for tt in range(8):
                pt = ps.tile([128, 512], F32, tag="mm")
                for dc in range(8):
                    nc.tensor.matmul(
                        pt[:, :256],
                        xcs[dc][:, tt * 128:(tt + 1) * 128],
                        wv2[dc // 4][:, dc % 4, :],
                        start=(dc == 0), stop=(dc == 7),
                    )
                vs = apool.tile([128, 256], F32, tag="vs")
                nc.vector.tensor_add(vs[:], pt[:, :256],
                                     PKs[:, PK_BVB:PK_BVB + 256])
                for g in range(2):
                    nc.scalar.activation(
                        V3[:, tt, g, 0:3:2, :],
                        vs[:, g * 128:(g + 1) * 128], SILU,
                        scale=PKs[:, PK_VM + tt:PK_VM + tt + 1])

            # ---- attention, rt-major; A-proj for each half interleaved ----
            AVs = big.tile([128, 2, 1024], F32R)  # [c%128, c//128, t]
            nUA = scratch.tile([128, 2, 1024], BF16, tag="nua")
            sstage = scratch.tile([1, 4, 512], F32, tag="sst")
            sqhs = []
            for rt in range(2):
                n_kt = 4 * rt + 4
                # head pairs interleaved: two independent mask+silu chains
                # feed the PE so it never outruns a single chain's latency
                for jp in (0, 2):
                    avp_d = {
                        j: psav.tile([128, 512], F32, tag="av",
                                     name=f"avp{rt}_{j}")
                        for j in (jp, jp + 1)
                    }
                    pend = []

                    def flush_one():
                        j_, att_, off_, kt_ = pend.pop(0)
                        nc.tensor.matmul(
                            avp_d[j_][:, off_:512],
                            V3[:, kt_, j_ // 2, (j_ % 2):(j_ % 2) + 2, :],
                            att_[:, off_:512],
                            start=(kt_ == 0), stop=(kt_ == n_kt - 1),
                            skip_group_check=True,
                        )

                    for kt in range(n_kt):
                        d0 = rt * 512 - kt * 128
                        off = max(0, -d0)   # causal-trim: i >= kt*128
                        cs = max(0, d0)
                        for j in (jp, jp + 1):
                            pb = (j % 2) * 64
                            ch = j // 2
                            qkp = ps.tile([128, 512], F32, tag="mm",
                                          name=f"qkp{rt}_{j}_{kt}")
                            nc.tensor.matmul(
                                qkp[:, off:512],
                                qk[pb:pb + 64, 2 + ch,
                                   kt * 128:(kt + 1) * 128],
                                qk[pb:pb + 64, ch,
                                   rt * 512 + off:(rt + 1) * 512],
                                start=True, stop=True,
                            )
                            # alpha*logits + mask window -> SBUF
                            asum = apool.tile([128, 512], F32, tag="asum")
                            nc.vector.scalar_tensor_tensor(
                                asum[:, off:512], qkp[:, off:512], alpha,
                                MTs[:, j, cs:cs + 512 - off], MULT, ADD)
                            att = apool.tile([128, 512], BF16, tag="att")
                            nc.scalar.activation(att[:, off:512],
                                                 asum[:, off:512], SILU)
                            pend.append((j, att, off, kt))
                            if len(pend) > 6:
                                flush_one()
                            if extra_work:
                                extra_work.pop(0)()
                    while pend:
                        flush_one()
                    for j in (jp, jp + 1):
                        pb = (j % 2) * 64
                        nc.vector.tensor_mul(
                            AVs[pb:pb + 64, j // 2, rt * 512:(rt + 1) * 512],
                            avp_d[j][pb:pb + 64, :],
                            PKs[pb:pb + 64,
                                PK_INVD + rt * 512:PK_INVD + (rt + 1) * 512])

                # nUA per g on DVE, sq on Pool in parallel; A-proj groups for
                # this half are queued as extra_work (rt0 drains inside rt1's
                # attention loop; rt1's leftovers drain below)
                half = slice(rt * 512, (rt + 1) * 512)
                sqh = scratch.tile([128, 2, 512], F32R, tag=f"sqh{rt}",
                                   name=f"sqh{rt}")
                sqhs.append(sqh)
                nc.gpsimd.tensor_mul(sqh[:], AVs[:, :, half], AVs[:, :, half])
                for g in range(2):
                    nc.vector.tensor_mul(nUA[:, g, half], AVs[:, g, half],
                                         gU[:, g, half])
                extra_work.extend(
                    make_agroup(nUA, dt_, rt) for dt_ in range(8))

            while extra_work:
                extra_work.pop(0)()

            # stats (tiny PE work, after all projection matmuls)
            for rt in range(2):
                half = slice(rt * 512, (rt + 1) * 512)
                for si in range(2):
                    sp = ps.tile([128, 512], F32, tag="mm",
                                 name=f"sp{rt}_{si}")
                    for g in range(2):
                        rhs = (AVs[:, g, half] if si == 0
                               else sqhs[rt][:, g, :])
                        nc.tensor.matmul(
                            sp[0:1, :], ONESs[:], rhs,
                            start=(g == 0), stop=(g == 1),
                        )
                    nc.vector.tensor_copy(
                        out=sstage[:, si * 2 + rt, :], in_=sp[0:1, :])

            nc.scalar.dma_start(SOUT[:], sstage[:])

    nc.compile()
    return nc


_NC_CACHE = {}


def _prep_in_maps(inputs):
    x = np.asarray(inputs["x"], np.float32)
    key_padding_mask = np.asarray(inputs["key_padding_mask"])
    W_in = np.asarray(inputs["W_in"], np.float32)
    b_in = np.asarray(inputs["b_in"], np.float32)
    W_out = np.asarray(inputs["W_out"], np.float32)
    gamma = np.asarray(inputs["gamma"], np.float32)
    beta = np.asarray(inputs["beta"], np.float32)
    rab_emb = np.asarray(inputs["rab_emb"], np.float32)

    lengths = (~key_padding_mask).sum(axis=1)  # valid keys per batch
    in_maps = []
    for core in range(8):
        b, hg = core // 4, core % 4
        sl = slice(hg * 256, hg * 256 + 256)
        Wu = W_in[0:1024][sl]
        Wv = W_in[1024:2048][sl]
        Wq = W_in[2048:3072][sl]
        Wk = W_in[3072:4096][sl]
        WC_np = np.concatenate([Wu, Wq, Wk], 0).T.reshape(8, 128, 768)
        WVC_np = Wv.T.reshape(2, 4, 128, 256).transpose(0, 2, 1, 3)
        XC_np = x[b].T.reshape(8, 128, 1024)
        WO_np = np.ascontiguousarray(
            W_out[:, sl].T.reshape(2, 128, 1024).transpose(1, 0, 2))
        L = int(lengths[b])
        denom = np.clip(np.minimum(np.arange(T) + 1, L), 1, None)
        heads = [4 * hg + jj for jj in range(H_PER)]
        MT_np = _toeplitz_mask_np(rab_emb, heads)

        PK_np = np.zeros((128, PK_N), np.float32)
        bqk = np.concatenate([b_in[2048:3072][sl], b_in[3072:4096][sl]])
        PK_np[:, PK_BQK:PK_BQK + 4] = bqk.reshape(4, 128).T
        PK_np[:, PK_BU:PK_BU + 2] = b_in[0:1024][sl].reshape(2, 128).T
        PK_np[:, PK_GAM:PK_GAM + 2] = gamma[sl].reshape(2, 128).T
        PK_np[:, PK_BET:PK_BET + 2] = beta[sl].reshape(2, 128).T
        PK_np[:, PK_VM:PK_VM + 8] = (
            np.arange(128)[:, None] + 128 * np.arange(8)[None, :] < L)
        PK_np[:, PK_INVD:PK_INVD + 1024] = (1.0 / denom)[None, :]
        PK_np[:, PK_BVB:PK_BVB + 256] = b_in[1024:2048][sl][None, :]

        in_maps.append({
            "XC": np.ascontiguousarray(XC_np).astype(BF16_NP),
            "WC": np.ascontiguousarray(WC_np).astype(BF16_NP),
            "WVC": np.ascontiguousarray(WVC_np).astype(BF16_NP),
            "WO": WO_np.astype(BF16_NP),
            "MT": MT_np.astype(BF16_NP),
            "PK": PK_np,
            "ONESP": np.ones((128, 1), np.float32),
        })
    return in_maps


def kernel(x, attention_mask, key_padding_mask, W_in, b_in, W_out, b_out,
           gamma, beta, rab_emb):
    global LAST_RESULTS
    x = np.asarray(x, np.float32)
    key_padding_mask = np.asarray(key_padding_mask)
    b_out = np.asarray(b_out, np.float32)
    beta = np.asarray(beta, np.float32)

    with_c = bool(np.any(beta != 0.0))
    if with_c not in _NC_CACHE:
        _NC_CACHE[with_c] = _build(with_c)
    nc = _NC_CACHE[with_c]

    in_maps = _prep_in_maps(dict(
        x=x, attention_mask=attention_mask, key_padding_mask=key_padding_mask,
        W_in=W_in, b_in=b_in, W_out=W_out, b_out=b_out, gamma=gamma,
        beta=beta, rab_emb=rab_emb))

    res = run_bass_kernel_spmd(nc, in_maps, list(range(8)))
    LAST_RESULTS = res

    out = np.empty((B, T, D), np.float32)
    for b in range(B):
        A = np.zeros((T, D), np.float64)
        Bm = np.zeros((T, D), np.float64)
        Cm = np.zeros((T, D), np.float64)
        s1 = np.zeros(T, np.float64)
        s2 = np.zeros(T, np.float64)
        for hg in range(4):
            r = res.results[4 * b + hg]
            A += r["AT"].reshape(1024, 1024).T.astype(np.float64)
            Bm += r["BT"].reshape(1024, 1024).T.astype(np.float64)
            if with_c:
                Cm += r["CT"].reshape(1024, 1024).T.astype(np.float64)
            s = r["SOUT"].reshape(4, 512)
            s1 += np.concatenate([s[0], s[1]]).astype(np.float64)
            s2 += np.concatenate([s[2], s[3]]).astype(np.float64)
        # s1, s2 already invd-scaled on device
        mu = s1 / D
        var = s2 / D - mu * mu
        rho = 1.0 / np.sqrt(var + LN_EPS)
        y = (rho[:, None] * A - (rho * mu)[:, None] * Bm + Cm
             + b_out[None, :].astype(np.float64) + x[b].astype(np.float64))
        out[b] = y.astype(np.float32)
    return out
